# revision 1
# baseline (speedup 1.0000x reference)
"""Trainium2 kernel for nn_ConvBlock (unfold -> max(thr) -> fold overlap-add -> crop).

Math: the unfold/max/fold pipeline collapses to a pointwise op,
    out[n,c,h,w] = sum_{(i,j) in V(h,w)} max(x[n,c,h,w], thr[c,3i+j])
where V is all 9 kernel offsets in the interior; at image edges the
row/col of offsets that would fall outside the output window drops out.
Using max(x,t) = t + relu(x-t):
    S9 = T_c + sum_k relu(x - t_ck)            (interior; T_c = sum_k thr[c,k])
edge corrections (inclusion-exclusion):
    h=0   : -(t6+t7+t8) - sum_{k in 6,7,8} relu(x - t_k)
    h=111 : -(t0+t1+t2) - sum_{k in 0,1,2} relu(x - t_k)
    w=0   : -(t2+t5+t8) - sum_{k in 2,5,8} relu(x - t_k)
    w=111 : -(t0+t3+t6) - sum_{k in 0,3,6} relu(x - t_k)
    corners add back the doubly-removed term: +max(x, t_k*), k* = 8,6,2,0.

Sharding: data-parallel, one batch sample per core (N=8 over 8 cores).
Per-core layout: partitions p = half*64 + c (h split in two 56-row halves),
free dim = 56*112 = 6272.

Self-contained: registers custom fused DVE ops at import time.
"""
import numpy as np

import concourse.bass as bass
import concourse.bacc as bacc
import concourse.mybir as mybir
import concourse.tile as tile
from concourse.bass_utils import run_bass_kernel_spmd

# ---------------------------------------------------------------- custom ops
from concourse.dve_ops import DveOp, OPS, CUSTOM_DVE_SPECS, _SUB_OPCODE_FOR_NAME, _CUSTOM_DVE_ROW_BASE
from concourse.dve_spec import (
    Spec, Src0, Src1, C0, C1, C3, relu, _spill_c3_to_src1, _has_src1, lower,
)
from concourse.dve_uop import DveOpSpec


def _register(name: str, spec: Spec, subdim: bool = False) -> DveOp:
    existing = {op.name: op for op in OPS}
    if name in existing:
        return existing[name]
    row = _CUSTOM_DVE_ROW_BASE + len(OPS)
    assert row < 0x20, "out of custom-DVE opcode rows"
    _SUB_OPCODE_FOR_NAME[name] = row
    shas = {}
    for ver in ("v3", "v4"):
        try:
            s = DveOpSpec(name=name, opcode=row, uops=lower(spec, ver=ver),
                          rd1_en=_has_src1(spec))
            shas[ver] = s.sha(ver)
        except Exception:
            pass
    op = DveOp(name, spec, subdim=subdim, uops_sha=shas)
    OPS.append(op)
    CUSTOM_DVE_SPECS[name] = spec
    return op


def _np_relu(v):
    return np.maximum(v, 0.0)


RELU3S = _register(
    "ANT_RELU3S",
    Spec(
        body=_spill_c3_to_src1(relu(Src0 - C0) + relu(Src0 - C1) + relu(Src0 - C3)),
        reference=lambda in0, in1, s0, s1, imm2:
            _np_relu(in0 - s0) + _np_relu(in0 - s1) + _np_relu(in0 - in1),
    ),
)
ACC_RELU2 = _register(
    "ANT_ACC_RELU2",
    Spec(
        body=Src1 + relu(Src0 - C0) + relu(Src0 - C1),
        reference=lambda in0, in1, s0, s1, imm2:
            in1 + _np_relu(in0 - s0) + _np_relu(in0 - s1),
    ),
)
SUB_SUB = _register(
    "ANT_SUB_SUB",
    Spec(
        body=Src0 - Src1 - C0,
        reference=lambda in0, in1, s0, s1, imm2: in0 - in1 - s0,
    ),
)
ACC_MAX1 = _register(
    "ANT_ACC_MAX1",
    Spec(
        body=Src1 + relu(Src0 - C0) + C0,
        reference=lambda in0, in1, s0, s1, imm2: in1 + _np_relu(in0 - s0) + s0,
    ),
)
# corner add-back with independent scalars so it can be masked per-partition:
# out = in1 + relu(x - s0) + s1   (s0=thr or +BIG, s1=thr or 0)
ACC_MAX1B = _register(
    "ANT_ACC_MAX1B",
    Spec(
        body=Src1 + relu(Src0 - C0) + C1,
        reference=lambda in0, in1, s0, s1, imm2: in1 + _np_relu(in0 - s0) + s1,
    ),
)

# ---------------------------------------------------------------- geometry
N_, C_, H_, W_ = 8, 64, 112, 112
HALF = H_ // 2                 # 56 rows per half
FD = HALF * W_                 # 6272 free-dim elements per partition
NT = 4                         # free-dim tiles
FDT = FD // NT                 # 1568 = 14 rows of 112
ROWS_T = FDT // W_             # 14
N_CORES = 8
F32 = mybir.dt.float32

_NC_CACHE = {}


def _build_nc(reps: int = 1):
    if reps in _NC_CACHE:
        return _NC_CACHE[reps]
    nc = bacc.Bacc("TRN2", debug=False, num_devices=N_CORES)
    x = nc.dram_tensor("x", [128, FD], F32, kind="ExternalInput")
    cst = nc.dram_tensor("cst", [128, 32], F32, kind="ExternalInput")
    y = nc.dram_tensor("y", [128, FD], F32, kind="ExternalOutput")

    with tile.TileContext(nc) as tc:
        with (
            tc.tile_pool(name="cpool", bufs=1) as cpool,
            tc.tile_pool(name="xpool", bufs=3) as xpool,
            tc.tile_pool(name="apool", bufs=3) as apool,
            tc.tile_pool(name="rpool", bufs=4) as rpool,
        ):
            cs = cpool.tile([128, 32], F32)
            nc.sync.dma_start(cs[:], cst[:])
            t = lambda k: cs[:, k:k + 1]

            for j in [jj for _ in range(reps) for jj in range(NT)]:
                xt = xpool.tile([128, FDT], F32)
                nc.sync.dma_start(xt[:], x[:, j * FDT:(j + 1) * FDT])
                a = apool.tile([128, FDT], F32)
                # interior sum of relus: 9 terms in 4 fused passes
                nc.vector._custom_dve(RELU3S, out=a[:], in0=xt[:], in1=t(2),
                                      s0=t(0), s1=t(1))
                nc.vector._custom_dve(ACC_RELU2, out=a[:], in0=xt[:], in1=a[:],
                                      s0=t(3), s1=t(4))
                nc.vector._custom_dve(ACC_RELU2, out=a[:], in0=xt[:], in1=a[:],
                                      s0=t(5), s1=t(6))
                nc.vector._custom_dve(ACC_RELU2, out=a[:], in0=xt[:], in1=a[:],
                                      s0=t(7), s1=t(8))

                x3 = xt[:].rearrange("p (r w) -> p r w", w=W_)
                a3 = a[:].rearrange("p (r w) -> p r w", w=W_)
                # w = 0 column: remove k in {2,5,8}
                rc0 = rpool.tile([128, ROWS_T], F32, tag="r")
                nc.vector._custom_dve(RELU3S, out=rc0[:], in0=x3[:, :, 0],
                                      in1=t(8), s0=t(2), s1=t(5))
                nc.vector._custom_dve(SUB_SUB, out=a3[:, :, 0], in0=a3[:, :, 0],
                                      in1=rc0[:], s0=t(12))
                # w = 111 column: remove k in {0,3,6}
                rc1 = rpool.tile([128, ROWS_T], F32, tag="r")
                nc.vector._custom_dve(RELU3S, out=rc1[:], in0=x3[:, :, W_ - 1],
                                      in1=t(6), s0=t(0), s1=t(3))
                nc.vector._custom_dve(SUB_SUB, out=a3[:, :, W_ - 1],
                                      in0=a3[:, :, W_ - 1], in1=rc1[:], s0=t(13))
                # NOTE: custom DVE ops misbehave at partition base != 0 in this
                # stack, so all edge corrections run on the full 128 partitions
                # with per-partition masked constants (+BIG threshold -> relu=0,
                # 0 offset -> no-op on the half where the row doesn't apply).
                if j == 0:
                    # h = 0 row (partitions 0:64 active, first 112 cols): remove k in {6,7,8}
                    rr = rpool.tile([128, W_], F32, tag="rrow")
                    nc.vector._custom_dve(RELU3S, out=rr[:], in0=xt[:, 0:W_],
                                          in1=cs[:, 16:17], s0=cs[:, 14:15],
                                          s1=cs[:, 15:16])
                    nc.vector._custom_dve(SUB_SUB, out=a[:, 0:W_],
                                          in0=a[:, 0:W_], in1=rr[:],
                                          s0=cs[:, 10:11])
                    # corners (0,0): +max(x,t8); (0,111): +max(x,t6)
                    nc.vector._custom_dve(ACC_MAX1B, out=a[:, 0:1],
                                          in0=xt[:, 0:1], in1=a[:, 0:1],
                                          s0=cs[:, 20:21], s1=cs[:, 21:22])
                    nc.vector._custom_dve(ACC_MAX1B, out=a[:, W_ - 1:W_],
                                          in0=xt[:, W_ - 1:W_],
                                          in1=a[:, W_ - 1:W_],
                                          s0=cs[:, 22:23], s1=cs[:, 23:24])
                if j == NT - 1:
                    # h = 111 row (partitions 64:128 active, last 112 cols): remove k in {0,1,2}
                    lo = FDT - W_
                    rr2 = rpool.tile([128, W_], F32, tag="rrow")
                    nc.vector._custom_dve(RELU3S, out=rr2[:], in0=xt[:, lo:FDT],
                                          in1=cs[:, 19:20], s0=cs[:, 17:18],
                                          s1=cs[:, 18:19])
                    nc.vector._custom_dve(SUB_SUB, out=a[:, lo:FDT],
                                          in0=a[:, lo:FDT], in1=rr2[:],
                                          s0=cs[:, 11:12])
                    # corners (111,0): +max(x,t2); (111,111): +max(x,t0)
                    nc.vector._custom_dve(ACC_MAX1B, out=a[:, lo:lo + 1],
                                          in0=xt[:, lo:lo + 1],
                                          in1=a[:, lo:lo + 1],
                                          s0=cs[:, 24:25], s1=cs[:, 25:26])
                    nc.vector._custom_dve(ACC_MAX1B, out=a[:, FDT - 1:FDT],
                                          in0=xt[:, FDT - 1:FDT],
                                          in1=a[:, FDT - 1:FDT],
                                          s0=cs[:, 26:27], s1=cs[:, 27:28])
                # + T (per-partition) on the scalar engine, then store
                nc.scalar.add(a[:], a[:], t(9))
                nc.sync.dma_start(y[:, j * FDT:(j + 1) * FDT], a[:])
    nc.compile()
    _NC_CACHE[reps] = nc
    return nc


def _make_consts(thr: np.ndarray) -> np.ndarray:
    # per-partition channel: p = half*64 + c  ->  c = p % 64
    BIG = np.float32(1e30)
    tpp = np.tile(thr, (2, 1)).astype(np.float32)        # (128, 9)
    top = np.arange(128) < 64                            # partitions holding h=0
    bot = ~top                                           # partitions holding h=111
    cst = np.zeros((128, 32), dtype=np.float32)
    cst[:, 0:9] = tpp
    cst[:, 9] = tpp.sum(axis=1)                          # T
    # row-correction constants, masked so ops are no-ops on the other half
    cst[:, 10] = np.where(top, tpp[:, 6] + tpp[:, 7] + tpp[:, 8], 0)  # T_top
    cst[:, 11] = np.where(bot, tpp[:, 0] + tpp[:, 1] + tpp[:, 2], 0)  # T_bot
    cst[:, 12] = tpp[:, 2] + tpp[:, 5] + tpp[:, 8]       # T_left  (w=0)
    cst[:, 13] = tpp[:, 0] + tpp[:, 3] + tpp[:, 6]       # T_right (w=111)
    cst[:, 14] = np.where(top, tpp[:, 6], BIG)           # h=0 relu thresholds
    cst[:, 15] = np.where(top, tpp[:, 7], BIG)
    cst[:, 16] = np.where(top, tpp[:, 8], BIG)
    cst[:, 17] = np.where(bot, tpp[:, 0], BIG)           # h=111 relu thresholds
    cst[:, 18] = np.where(bot, tpp[:, 1], BIG)
    cst[:, 19] = np.where(bot, tpp[:, 2], BIG)
    # corner add-backs: (C0: thr or +BIG, C1: thr or 0)
    cst[:, 20] = np.where(top, tpp[:, 8], BIG)           # (0,0)
    cst[:, 21] = np.where(top, tpp[:, 8], 0)
    cst[:, 22] = np.where(top, tpp[:, 6], BIG)           # (0,111)
    cst[:, 23] = np.where(top, tpp[:, 6], 0)
    cst[:, 24] = np.where(bot, tpp[:, 2], BIG)           # (111,0)
    cst[:, 25] = np.where(bot, tpp[:, 2], 0)
    cst[:, 26] = np.where(bot, tpp[:, 0], BIG)           # (111,111)
    cst[:, 27] = np.where(bot, tpp[:, 0], 0)
    return cst


def kernel(x: np.ndarray, thr: np.ndarray) -> np.ndarray:
    x = np.ascontiguousarray(x, dtype=np.float32)
    thr = np.ascontiguousarray(thr, dtype=np.float32)
    assert x.shape == (N_, C_, H_, W_) and thr.shape == (C_, 9)
    nc = _build_nc()
    cst = _make_consts(thr)
    in_maps = []
    for n in range(N_CORES):
        xs = x[n].reshape(C_, 2, FD).transpose(1, 0, 2).reshape(128, FD)
        in_maps.append({"x": np.ascontiguousarray(xs), "cst": cst})
    res = run_bass_kernel_spmd(nc, in_maps, core_ids=list(range(N_CORES)))
    out = np.empty((N_, C_, H_, W_), dtype=np.float32)
    for n in range(N_CORES):
        yn = res.results[n]["y"]
        out[n] = (yn.reshape(2, C_, FD).transpose(1, 0, 2)
                  .reshape(C_, H_, W_))
    return out

